# revision 1
# baseline (speedup 1.0000x reference)
"""Trainium2 Bass kernel for nn_MISA (dense_transformer, data-parallel over 8 cores).

Layout: feature-major activations [feat_part=128, mtile, batch_cols] per core.
Batch 4096 -> 512 per core -> two passes of 256 columns.
All matmuls bf16 (fp32 PSUM accumulation); LN/softmax internals fp32.
Host pre-transposes weights to [in,out] and inputs to feat-major.

Structural simplifications (exact, not approximations):
- attention with all-equal keys/values (q/k/v = broadcast joint row) is the
  identity on v: cross_tj == cross_sj == out_proj4(v_proj4(joint)).
- mean over query positions commutes with out_proj and with A@V, so the six
  cross outputs never materialize per-query outputs (abar-weighted V only).
- all-equal queries (j as q): single query row, output equals its mean.
"""
import sys, math
from contextlib import ExitStack
sys.path.insert(0, "/opt/trn_rl_repo")

import numpy as np
import ml_dtypes

import concourse.bass as bass
import concourse.mybir as mybir
from concourse import bacc
import concourse.tile as tile
from concourse import bass_utils

F32 = mybir.dt.float32
BF16 = mybir.dt.bfloat16
AF = mybir.ActivationFunctionType
ALU = mybir.AluOpType
BF = ml_dtypes.bfloat16

H = 8
E = 4
HD = 1024
B = 4096
NCORES = 8
BC = B // NCORES          # 512 batch per core
NP = 2                    # passes per core
N = BC // NP              # 256 batch cols per pass
EPS = 1e-5


def _bias_cols(b):
    # [M] -> [128, M//128]: column m = per-partition bias of m-tile m
    return np.ascontiguousarray(np.asarray(b, np.float32).reshape(-1, 128).T)


def build(res_w: float):
    nc = bacc.Bacc("TRN2", target_bir_lowering=False, debug=False)

    def din(name, shape, dt):
        return nc.dram_tensor(name, list(shape), dt, kind="ExternalInput").ap()

    xt_d = din("xt", (128, 8, BC), F32)
    xs_d = din("xs", (128, 8, BC), F32)
    wexp = [din(f"wexp{j}", (HD, E * HD), BF16) for j in range(2)]
    bexp = [din(f"bexp{j}", (128, 32), F32) for j in range(2)]
    wqkv = [din(f"wqkv{i}", (HD, 3 * HD), BF16) for i in range(5)]
    bqkv = [din(f"bqkv{i}", (128, 24), F32) for i in range(5)]
    wout = [din(f"wout{i}", (HD, HD), BF16) for i in range(5)]
    bout = [din(f"bout{i}", (128, 8), F32) for i in range(5)]
    wjoint = din("wjoint", (2 * HD, HD), BF16)
    bjoint = din("bjoint", (128, 8), F32)
    wgate = [din(f"wgate{g}", (2 * HD, HD), BF16) for g in range(3)]
    bgate = [din(f"bgate{g}", (128, 8), F32) for g in range(3)]
    wo1 = din("wo1", (6 * HD, 2 * HD), BF16)
    bo1 = din("bo1", (128, 16), F32)
    wo2 = din("wo2", (2 * HD, HD), BF16)
    bo2 = din("bo2", (128, 8), F32)
    lng = [din(f"lng{i}", (128, 8), F32) for i in range(3)]
    lnb = [din(f"lnb{i}", (128, 8), F32) for i in range(3)]
    sel_d = din("sel_c", (8, 8 * 128), BF16)
    o32_d = din("o32_c", (128, 64), BF16)
    y_d = nc.dram_tensor("y", [128, 8, BC], F32, kind="ExternalOutput").ap()

    with tile.TileContext(nc) as tc, ExitStack() as ctx:
        P = lambda **kw: ctx.enter_context(tc.tile_pool(**kw))
        cst = P(name="cst", bufs=1)
        wtp = P(name="wtp", bufs=3)
        mmp = P(name="mmp", bufs=4, space="PSUM")   # 4 x [128,N] banks
        scp = P(name="scp", bufs=1, space="PSUM")   # [8,4,N] = 2 banks
        brp = P(name="brp", bufs=2, space="PSUM")   # 2 x [128,N] banks
        expp = P(name="expp", bufs=1)   # tag exp: xp, also h1 (disjoint lifetime)
        enhp = P(name="enhp", bufs=2)
        qkvp = P(name="qkvp", bufs=2)   # K,V tiles
        qep = P(name="qep", bufs=2)     # per-position Q
        actp = P(name="actp", bufs=3)
        meanp = P(name="meanp", bufs=6)
        gatep = P(name="gatep", bufs=2)
        qkp = P(name="qkp", bufs=3)     # qk products + av tmp
        avp = P(name="avp", bufs=2)     # av acc, also x bf16 casts
        tresp = P(name="tresp", bufs=2)
        xinp = P(name="xinp", bufs=1)
        smp = P(name="smp", bufs=2)
        smbp = P(name="smbp", bufs=4)
        smrp = P(name="smrp", bufs=6)
        smabp = P(name="smabp", bufs=4)
        yp = P(name="yp", bufs=1)

        _tc = [0]
        def T(pool, shape, dtype, tag):
            _tc[0] += 1
            return pool.tile(shape, dtype, tag=tag, name=f"{tag}_{_tc[0]}")

        ones_b = T(cst, [128, 1], BF16, "ones_b")
        nc.any.memset(ones_b[:], 1.0)
        onerow_f = T(cst, [1, 128], F32, "onerow_f")
        nc.any.memset(onerow_f[:], 1.0)
        sel = T(cst, [8, 8 * 128], BF16, "sel")
        nc.sync.dma_start(out=sel[:], in_=sel_d)
        o32 = T(cst, [128, 64], BF16, "o32")   # all-ones column at col 32
        nc.sync.dma_start(out=o32[:], in_=o32_d)
        eps_t = T(cst, [1, 1], F32, "eps_t")
        nc.any.memset(eps_t[:], EPS)

        def ctile(name, ap):
            t = cst.tile(list(ap.shape), ap.dtype, tag=name)
            nc.sync.dma_start(out=t[:], in_=ap)
            return t

        bexp_t = [ctile(f"bexp{j}", bexp[j]) for j in range(2)]
        bqkv_t = [ctile(f"bqkv{i}", bqkv[i]) for i in range(5)]
        bout_t = [ctile(f"bout{i}", bout[i]) for i in range(5)]
        bjoint_t = ctile("bjoint", bjoint)
        bgate_t = [ctile(f"bgate{g}", bgate[g]) for g in range(3)]
        bo1_t = ctile("bo1", bo1)
        bo2_t = ctile("bo2", bo2)
        lng_t = [ctile(f"lng{i}", lng[i]) for i in range(3)]
        lnb_t = [ctile(f"lnb{i}", lnb[i]) for i in range(3)]

        def proj(w_ap, M, K, srcs, evict, wcol0=0):
            """psum[mt] += w[kt, mt-block].T @ srcs(kt); then evict(mi, psum).
            M in groups of 4 m-tiles to bound PSUM usage."""
            nmt = M // 128
            nkt = K // 128
            for g0 in range(0, nmt, 4):
                gm = min(4, nmt - g0)
                psums = [T(mmp, [128, N], F32, "mm") for _ in range(gm)]
                for kt in range(nkt):
                    wt = T(wtp, [128, 4 * 128], BF16, "wt")
                    nc.sync.dma_start(
                        out=wt[:, : gm * 128],
                        in_=w_ap[kt * 128 : (kt + 1) * 128,
                                 wcol0 + g0 * 128 : wcol0 + (g0 + gm) * 128],
                    )
                    s = srcs(kt)
                    for mi in range(gm):
                        nc.tensor.matmul(
                            psums[mi][:], wt[:, mi * 128 : (mi + 1) * 128], s,
                            start=(kt == 0), stop=(kt == nkt - 1),
                        )
                for mi in range(gm):
                    evict(g0 + mi, psums[mi])

        def ev_dve(dst3, btile, bcol0=0, colmap=None):
            def _ev(mi, ps):
                col = mi if colmap is None else colmap(mi)
                nc.vector.tensor_scalar_add(
                    out=dst3[:, col, :], in0=ps[:],
                    scalar1=btile[:, bcol0 + mi : bcol0 + mi + 1])
            return _ev

        def ev_act(dst3, btile, func):
            def _ev(mi, ps):
                nc.scalar.activation(dst3[:, mi, :], ps[:], func,
                                     bias=btile[:, mi : mi + 1])
            return _ev

        def scores_all(q3, k4, e1):
            """psum [32,N]: rows e2*8+h = q[e1] . k[e2] per head (q pre-scaled)."""
            sp = T(scp, [8, 4, N], F32, "sc")
            q = q3[:, e1 * 8 : (e1 + 1) * 8, :] if q3.shape[1] == 32 else q3[:]
            for e2 in range(4):
                p = T(qkp, [128, 8, N], BF16, "qk")
                nc.vector.tensor_tensor(
                    out=p[:], in0=q, in1=k4[:, e2 * 8 : (e2 + 1) * 8, :], op=ALU.mult)
                for kt in range(8):
                    nc.tensor.matmul(sp[:, e2, :], o32[:, 32 - kt : 40 - kt],
                                     p[:, kt, :], start=(kt == 0), stop=(kt == 7))
            return sp

        def softmax_tiles(sp):
            """sp [32,N] psum scores -> 4 bf16 [8,N] attention-weight tiles."""
            et = T(smp, [8, 4, N], F32, "sm")
            nc.scalar.activation(et[:], sp[:], AF.Exp)
            d = T(smp, [8, N], F32, "smd")
            nc.vector.tensor_add(out=d[:], in0=et[:, 0, :], in1=et[:, 1, :])
            for e2 in (2, 3):
                nc.vector.tensor_add(out=d[:], in0=d[:], in1=et[:, e2, :])
            r = T(smp, [8, N], F32, "smd")
            nc.vector.reciprocal(r[:], d[:])
            outs = []
            for e2 in range(4):
                a = T(smbp, [8, N], BF16, "smb")
                nc.vector.tensor_tensor(out=a[:], in0=et[:, e2, :], in1=r[:],
                                        op=ALU.mult)
                outs.append(a)
            return outs

        def av_accum(a_list, v4):
            """acc [128,8,N] bf16 = sum_e2 broadcast_heads(a_list[e2]) * V[e2]."""
            acc = T(avp, [128, 8, N], BF16, "av")
            t3 = T(qkp, [128, 8, N], BF16, "qk")
            for e2 in range(4):
                dst = acc if e2 == 0 else t3
                for mt in range(8):
                    bp = T(brp, [128, N], F32, "br")
                    nc.tensor.matmul(bp[:], sel[:, mt * 128 : (mt + 1) * 128],
                                     a_list[e2][:], start=True, stop=True)
                    nc.vector.tensor_tensor(
                        out=dst[:, mt, :], in0=bp[:],
                        in1=v4[:, e2 * 8 + mt, :], op=ALU.mult)
                if e2 > 0:
                    nc.vector.tensor_add(out=acc[:], in0=acc[:], in1=t3[:])
            return acc

        def ln_norm(x3, g_t, b_t, dst3, dcol0):
            """LayerNorm over the 1024 feats (partitions x 8 mtiles) of x3 (f32)."""
            sq = T(qkp, [128, 8, N], BF16, "qk")
            nc.vector.tensor_tensor(out=sq[:], in0=x3[:], in1=x3[:], op=ALU.mult)
            st_s = T(scp, [1, N], F32, "sc")
            for kt in range(8):
                nc.tensor.matmul(st_s[:], ones_b[:], x3[:, kt, :],
                                 start=(kt == 0), stop=(kt == 7))
            st_q = T(scp, [1, N], F32, "sc")
            for kt in range(8):
                nc.tensor.matmul(st_q[:], ones_b[:], sq[:, kt, :],
                                 start=(kt == 0), stop=(kt == 7))
            mean = T(smrp, [1, N], F32, "smr")
            nc.scalar.mul(mean[:], st_s[:], 1.0 / HD)
            msq = T(smrp, [1, N], F32, "smr")
            nc.scalar.mul(msq[:], st_q[:], 1.0 / HD)
            m2 = T(smrp, [1, N], F32, "smr")
            nc.vector.tensor_tensor(out=m2[:], in0=mean[:], in1=mean[:], op=ALU.mult)
            var = T(smrp, [1, N], F32, "smr")
            nc.vector.tensor_tensor(out=var[:], in0=msq[:], in1=m2[:],
                                    op=ALU.subtract)
            std = T(smrp, [1, N], F32, "smr")
            nc.scalar.activation(std[:], var[:], AF.Sqrt, bias=eps_t[:])
            rstd = T(smrp, [1, N], F32, "smr")
            nc.vector.reciprocal(rstd[:], std[:])
            mb = T(brp, [128, N], F32, "br")
            nc.tensor.matmul(mb[:], onerow_f[:], mean[:], start=True, stop=True)
            rb = T(brp, [128, N], F32, "br")
            nc.tensor.matmul(rb[:], onerow_f[:], rstd[:], start=True, stop=True)
            for mt in range(8):
                nc.vector.tensor_tensor(out=x3[:, mt, :], in0=x3[:, mt, :],
                                        in1=mb[:], op=ALU.subtract)
                nc.vector.tensor_tensor(out=x3[:, mt, :], in0=x3[:, mt, :],
                                        in1=rb[:], op=ALU.mult)
                nc.vector.tensor_scalar(
                    out=dst3[:, dcol0 + mt, :], in0=x3[:, mt, :],
                    scalar1=g_t[:, mt : mt + 1], scalar2=b_t[:, mt : mt + 1],
                    op0=ALU.mult, op1=ALU.add)

        for c in range(NP):
            bs = slice(c * N, (c + 1) * N)

            xt_f = T(xinp, [128, 8, N], F32, "xin")
            nc.sync.dma_start(out=xt_f[:], in_=xt_d[:, :, bs])
            xt_b = T(avp, [128, 8, N], BF16, "av")
            nc.vector.tensor_copy(out=xt_b[:], in_=xt_f[:])
            xs_f = T(xinp, [128, 8, N], F32, "xin")
            nc.sync.dma_start(out=xs_f[:], in_=xs_d[:, :, bs])
            xs_b = T(avp, [128, 8, N], BF16, "av")
            nc.vector.tensor_copy(out=xs_b[:], in_=xs_f[:])

            def run_self(x_b, j, sum_dst, enh_dst):
                """expand -> self-MHA(j) -> +resid -> LN -> enh_dst; sum_dst=sum_e."""
                xp = T(expp, [128, 32, N], BF16, "exp")
                proj(wexp[j], 4 * HD, HD, lambda kt: x_b[:, kt, :],
                     ev_dve(xp, bexp_t[j]))
                k4 = T(qkvp, [128, 32, N], BF16, "qkv")
                v4 = T(qkvp, [128, 32, N], BF16, "qkv")
                for e in range(4):
                    def evkv(mj, ps, e=e):
                        dst = k4 if mj < 8 else v4
                        nc.vector.tensor_scalar_add(
                            out=dst[:, e * 8 + mj % 8, :], in0=ps[:],
                            scalar1=bqkv_t[j][:, 8 + mj : 9 + mj])
                    proj(wqkv[j], 2 * HD, HD,
                         lambda kt, e=e: xp[:, e * 8 + kt, :], evkv, wcol0=HD)
                for e1 in range(4):
                    qe = T(qep, [128, 8, N], BF16, "qe")
                    proj(wqkv[j], HD, HD,
                         lambda kt, e1=e1: xp[:, e1 * 8 + kt, :],
                         ev_dve(qe, bqkv_t[j]))
                    a_l = softmax_tiles(scores_all(qe, k4, e1))
                    acc = av_accum(a_l, v4)
                    tres = T(tresp, [128, 8, N], BF16, "tres")
                    def evo(mj, ps, e1=e1, tres=tres):
                        nc.vector.tensor_scalar_add(
                            out=tres[:, mj, :], in0=ps[:],
                            scalar1=bout_t[j][:, mj : mj + 1])
                        nc.vector.tensor_add(
                            out=tres[:, mj, :], in0=tres[:, mj, :],
                            in1=xp[:, e1 * 8 + mj, :])
                    proj(wout[j], HD, HD, lambda kt: acc[:, kt, :], evo)
                    ln_norm(tres, lng_t[j], lnb_t[j], enh_dst, e1 * 8)
                nc.vector.tensor_add(out=sum_dst[:], in0=enh_dst[:, 0:8, :],
                                     in1=enh_dst[:, 8:16, :])
                t2 = T(qkp, [128, 8, N], BF16, "qk")
                nc.vector.tensor_add(out=t2[:], in0=enh_dst[:, 16:24, :],
                                     in1=enh_dst[:, 24:32, :])
                nc.vector.tensor_add(out=sum_dst[:], in0=sum_dst[:], in1=t2[:])

            t_enh = T(enhp, [128, 32, N], BF16, "enh")
            s_enh = T(enhp, [128, 32, N], BF16, "enh")
            sum_t = T(actp, [128, 8, N], BF16, "act")
            sum_s = T(actp, [128, 8, N], BF16, "act")
            run_self(xt_b, 0, sum_t, t_enh)
            run_self(xs_b, 1, sum_s, s_enh)

            joint = T(actp, [128, 8, N], BF16, "act")
            proj(wjoint, HD, 2 * HD,
                 lambda kt: sum_t[:, kt, :] if kt < 8 else sum_s[:, kt - 8, :],
                 ev_dve(joint, bjoint_t))

            def run_cross(mi, qsrc4, kvsrc4, dst):
                """cross-attn, output mean over query positions -> dst [128,8,N]."""
                k4 = T(qkvp, [128, 32, N], BF16, "qkv")
                v4 = T(qkvp, [128, 32, N], BF16, "qkv")
                for e in range(4):
                    def evkv(mj, ps, e=e):
                        dst_ = k4 if mj < 8 else v4
                        nc.vector.tensor_scalar_add(
                            out=dst_[:, e * 8 + mj % 8, :], in0=ps[:],
                            scalar1=bqkv_t[mi][:, 8 + mj : 9 + mj])
                    proj(wqkv[mi], 2 * HD, HD,
                         lambda kt, e=e: kvsrc4[:, e * 8 + kt, :], evkv, wcol0=HD)
                abar = [None] * 4
                for e1 in range(4):
                    qe = T(qep, [128, 8, N], BF16, "qe")
                    proj(wqkv[mi], HD, HD,
                         lambda kt, e1=e1: qsrc4[:, e1 * 8 + kt, :],
                         ev_dve(qe, bqkv_t[mi]))
                    a_l = softmax_tiles(scores_all(qe, k4, e1))
                    for e2 in range(4):
                        if e1 == 0:
                            ab = T(smabp, [8, N], F32, "smab")
                            nc.vector.tensor_copy(out=ab[:], in_=a_l[e2][:])
                            abar[e2] = ab
                        else:
                            nc.vector.tensor_add(out=abar[e2][:], in0=abar[e2][:],
                                                 in1=a_l[e2][:])
                abar_b = []
                for e2 in range(4):
                    ab = T(smbp, [8, N], BF16, "smb")
                    nc.scalar.mul(ab[:], abar[e2][:], 0.25)
                    abar_b.append(ab)
                acc = av_accum(abar_b, v4)
                proj(wout[mi], HD, HD, lambda kt: acc[:, kt, :],
                     ev_dve(dst, bout_t[mi]))

            mts = T(meanp, [128, 8, N], BF16, "mean")
            mst = T(meanp, [128, 8, N], BF16, "mean")
            run_cross(2, t_enh, s_enh, mts)
            run_cross(3, s_enh, t_enh, mst)

            # mha4 group: mtj == msj == Wout4 @ (Wv4 @ joint + bv4) + bout4
            vj = T(actp, [128, 8, N], BF16, "act")
            proj(wqkv[4], HD, HD, lambda kt: joint[:, kt, :],
                 ev_dve(vj, bqkv_t[4], bcol0=16), wcol0=2 * HD)
            mtj = T(meanp, [128, 8, N], BF16, "mean")
            proj(wout[4], HD, HD, lambda kt: vj[:, kt, :],
                 ev_dve(mtj, bout_t[4]))
            qj = T(actp, [128, 8, N], BF16, "act")
            proj(wqkv[4], HD, HD, lambda kt: joint[:, kt, :],
                 ev_dve(qj, bqkv_t[4]))

            def run_jx(kvsrc4, dst):
                """single-query cross-attn (q = joint row) -> dst [128,8,N]."""
                k4 = T(qkvp, [128, 32, N], BF16, "qkv")
                v4 = T(qkvp, [128, 32, N], BF16, "qkv")
                for e in range(4):
                    def evkv(mj, ps, e=e):
                        dst_ = k4 if mj < 8 else v4
                        nc.vector.tensor_scalar_add(
                            out=dst_[:, e * 8 + mj % 8, :], in0=ps[:],
                            scalar1=bqkv_t[4][:, 8 + mj : 9 + mj])
                    proj(wqkv[4], 2 * HD, HD,
                         lambda kt, e=e: kvsrc4[:, e * 8 + kt, :], evkv, wcol0=HD)
                a_l = softmax_tiles(scores_all(qj, k4, 0))
                acc = av_accum(a_l, v4)
                proj(wout[4], HD, HD, lambda kt: acc[:, kt, :],
                     ev_dve(dst, bout_t[4]))

            mjt = T(meanp, [128, 8, N], BF16, "mean")
            mjs = T(meanp, [128, 8, N], BF16, "mean")
            run_jx(t_enh, mjt)
            run_jx(s_enh, mjs)

            def run_gate(g, in_a, in_b):
                gt = T(gatep, [128, 8, N], BF16, "gate")
                proj(wgate[g], HD, 2 * HD,
                     lambda kt: in_a[:, kt, :] if kt < 8 else in_b[:, kt - 8, :],
                     ev_act(gt, bgate_t[g], AF.Sigmoid))
                return gt

            gate_t = run_gate(0, mts, mtj)
            gate_s = run_gate(1, mst, mtj)
            gate_j = run_gate(2, mjt, mjs)

            f2 = T(meanp, [128, 8, N], BF16, "mean")
            nc.vector.tensor_tensor(out=f2[:], in0=gate_t[:], in1=mtj[:], op=ALU.mult)
            nc.vector.tensor_tensor(out=mts[:], in0=gate_t[:], in1=mts[:], op=ALU.mult)
            nc.vector.tensor_tensor(out=mst[:], in0=gate_s[:], in1=mst[:], op=ALU.mult)
            nc.vector.tensor_tensor(out=mtj[:], in0=gate_s[:], in1=mtj[:], op=ALU.mult)
            nc.vector.tensor_tensor(out=mjt[:], in0=gate_j[:], in1=mjt[:], op=ALU.mult)
            nc.vector.tensor_tensor(out=mjs[:], in0=gate_j[:], in1=mjs[:], op=ALU.mult)
            fs = [mts, mst, f2, mtj, mjt, mjs]

            h1 = T(expp, [128, 16, N], BF16, "exp")
            proj(wo1, 2 * HD, 6 * HD, lambda kt: fs[kt // 8][:, kt % 8, :],
                 ev_act(h1, bo1_t, AF.Relu))
            h2 = T(tresp, [128, 8, N], BF16, "tres")
            proj(wo2, HD, 2 * HD, lambda kt: h1[:, kt, :], ev_dve(h2, bo2_t))

            yt = T(yp, [128, 8, N], F32, "y")
            ln_norm(h2, lng_t[2], lnb_t[2], yt, 0)
            nc.scalar.mul(yt[:], yt[:], res_w)
            cres = (1.0 - res_w) * 0.5
            for xd in (xt_d, xs_d):
                xr = T(xinp, [128, 8, N], F32, "xin")
                nc.sync.dma_start(out=xr[:], in_=xd[:, :, bs])
                nc.scalar.mul(xr[:], xr[:], cres)
                nc.vector.tensor_add(out=yt[:], in0=yt[:], in1=xr[:])
            nc.sync.dma_start(out=y_d[:, :, bs], in_=yt[:])

    nc.compile()
    return nc


def _sel_const():
    s = np.zeros((8, 8 * 128), np.float32)
    for mt in range(8):
        s[mt, mt * 128 : (mt + 1) * 128] = 1.0
    return s.astype(BF)


def _o32_const():
    o = np.zeros((128, 64), np.float32)
    o[:, 32] = 1.0
    return o.astype(BF)


def _prep_inputs(i):
    res_w = float(np.asarray(i["res_w"]).reshape(-1)[0])
    sc = 1.0 / math.sqrt(128.0)

    def bf(x):
        return np.ascontiguousarray(np.asarray(x, np.float32).T).astype(BF)

    shared = {
        "wexp0": bf(i["exp_t_w"]), "wexp1": bf(i["exp_s_w"]),
        "bexp0": _bias_cols(np.asarray(i["exp_t_b"]) + np.asarray(i["pos_enc"]).reshape(-1)),
        "bexp1": _bias_cols(np.asarray(i["exp_s_b"]) + np.asarray(i["pos_enc"]).reshape(-1)),
        "wjoint": bf(np.asarray(i["joint_w"], np.float32) * 0.25),
        "bjoint": _bias_cols(i["joint_b"]),
        "wo1": bf(i["out1_w"]), "bo1": _bias_cols(i["out1_b"]),
        "wo2": bf(i["out2_w"]), "bo2": _bias_cols(i["out2_b"]),
        "sel_c": _sel_const(), "o32_c": _o32_const(),
    }
    for g in range(3):
        shared[f"wgate{g}"] = bf(i["gate_w"][g])
        shared[f"bgate{g}"] = _bias_cols(i["gate_b"][g])
    for m in range(5):
        w = np.asarray(i["mha_in_w"][m], np.float32).copy()
        b = np.asarray(i["mha_in_b"][m], np.float32).copy()
        w[:HD] *= sc
        b[:HD] *= sc
        shared[f"wqkv{m}"] = bf(w)
        shared[f"bqkv{m}"] = _bias_cols(b)
        shared[f"wout{m}"] = bf(i["mha_out_w"][m])
        shared[f"bout{m}"] = _bias_cols(i["mha_out_b"][m])
    for ln in range(3):
        shared[f"lng{ln}"] = _bias_cols(i["ln_g"][ln])
        shared[f"lnb{ln}"] = _bias_cols(i["ln_b"][ln])

    def shard_x(x, c):
        xc = np.asarray(x, np.float32)[c * BC : (c + 1) * BC, 0, :]  # [512,1024]
        return np.ascontiguousarray(xc.T.reshape(8, 128, BC).transpose(1, 0, 2))

    in_maps = []
    for c in range(NCORES):
        m = dict(shared)
        m["xt"] = shard_x(i["temporal_features"], c)
        m["xs"] = shard_x(i["spatial_features"], c)
        in_maps.append(m)
    return res_w, in_maps


def kernel(**inputs):
    res_w, in_maps = _prep_inputs(inputs)
    nc = build(res_w)
    res = bass_utils.run_bass_kernel_spmd(nc, in_maps, core_ids=list(range(NCORES)))
    outs = []
    for c in range(NCORES):
        y = res.results[c]["y"]                                   # [128,8,512]
        outs.append(np.asarray(y).transpose(1, 0, 2).reshape(HD, BC).T)
    return np.concatenate(outs, 0)[:, None, :].astype(np.float32)



# revision 10
# speedup vs baseline: 1.6941x; 1.6941x over previous
"""Trainium2 Bass kernel for nn_MISA (dense_transformer, data-parallel over 8 cores).

Layout: feature-major activations [feat_part=128, mtile, batch_cols] per core.
Batch 4096 -> 512 per core -> two passes of 256 columns.
All matmuls bf16 (fp32 PSUM accumulation); LN/softmax internals fp32.

v2 (DMA restructure): weights live in DRAM as [128, K/128, M] and each
weight tile is DMA'd exactly once per pass:
- projS shares one stationary weight load across the 4 expand positions
  (pairs of positions ride one matmul: moving [128,2,N] -> PSUM [128,512]).
- PSUM evictions run on the Scalar engine (Identity+bias) to unload DVE.
- self-attn residual is folded into the out-proj PSUM via an identity matmul,
  and the out-proj evicts in place over the expand tile.
- output is written bf16 and cast to f32 on host.

Structural simplifications (exact, not approximations):
- attention with all-equal keys/values (q/k/v = broadcast joint row) is the
  identity on v: cross_tj == cross_sj == out_proj4(v_proj4(joint)).
- mean over query positions commutes with out_proj and with A@V, so the six
  cross outputs never materialize per-query outputs (abar-weighted V only).
- all-equal queries (j as q): single query row, output equals its mean.
"""
import sys, math
from contextlib import ExitStack
sys.path.insert(0, "/opt/trn_rl_repo")

import numpy as np
import ml_dtypes

import concourse.bass as bass
import concourse.mybir as mybir
from concourse import bacc
import concourse.tile as tile
from concourse import bass_utils

F32 = mybir.dt.float32
BF16 = mybir.dt.bfloat16
AF = mybir.ActivationFunctionType
ALU = mybir.AluOpType
BF = ml_dtypes.bfloat16

H = 8
E = 4
HD = 1024
B = 4096
NCORES = 8
BC = B // NCORES          # 512 batch per core
NP = 2                    # passes per core
N = BC // NP              # 256 batch cols per pass
EPS = 1e-5


def _bias_cols(b):
    # [M] -> [128, M//128]: column m = per-partition bias of m-tile m
    return np.ascontiguousarray(np.asarray(b, np.float32).reshape(-1, 128).T)


def build(res_w: float):
    nc = bacc.Bacc("TRN2", target_bir_lowering=False, debug=False)

    def din(name, shape, dt):
        return nc.dram_tensor(name, list(shape), dt, kind="ExternalInput").ap()

    xt_d = din("xt", (128, 8, BC), F32)
    xs_d = din("xs", (128, 8, BC), F32)
    # weights as [128, K/128, M] (partition = row within k-tile)
    wexp = [din(f"wexp{j}", (128, 8, E * HD), BF16) for j in range(2)]
    bexp = [din(f"bexp{j}", (128, 32), F32) for j in range(2)]
    wqkv = [din(f"wqkv{i}", (128, 8, 3 * HD), BF16) for i in range(5)]
    bqkv = [din(f"bqkv{i}", (128, 24), F32) for i in range(5)]
    wout = [din(f"wout{i}", (128, 8, HD), BF16) for i in range(5)]
    bout = [din(f"bout{i}", (128, 8), F32) for i in range(5)]
    wjoint = din("wjoint", (128, 16, HD), BF16)
    bjoint = din("bjoint", (128, 8), F32)
    wgate = [din(f"wgate{g}", (128, 16, HD), BF16) for g in range(3)]
    bgate = [din(f"bgate{g}", (128, 8), F32) for g in range(3)]
    wo1 = din("wo1", (128, 48, 2 * HD), BF16)
    bo1 = din("bo1", (128, 16), F32)
    wo2 = din("wo2", (128, 16, HD), BF16)
    bo2 = din("bo2", (128, 8), F32)
    lng = [din(f"lng{i}", (128, 8), F32) for i in range(3)]
    lnb = [din(f"lnb{i}", (128, 8), F32) for i in range(3)]
    sel_d = din("sel_c", (8, 8 * 128), BF16)
    o32_d = din("o32_c", (128, 64), BF16)
    i128_d = din("i128_c", (128, 128), BF16)
    y_d = nc.dram_tensor("y", [128, 8, BC], BF16, kind="ExternalOutput").ap()

    with tile.TileContext(nc) as tc, ExitStack() as ctx:
        P = lambda **kw: ctx.enter_context(tc.tile_pool(**kw))
        cst = P(name="cst", bufs=1)
        wgp = P(name="wgp", bufs=2)                 # [128,8,256] weight blocks
        mmp = P(name="mmp", bufs=4, space="PSUM")   # 4 x [128,512] banks
        scp = P(name="scp", bufs=1, space="PSUM")   # [8,4,N] = 2 banks
        brp = P(name="brp", bufs=2, space="PSUM")   # 2 x [128,N] banks
        expp = P(name="expp", bufs=1)   # xp [128,8,4,N]; also h1
        enhp = P(name="enhp", bufs=2)   # t_enh, s_enh [128,8,4,N]
        qkvp = P(name="qkvp", bufs=2)   # k4, v4 [128,8,4,N]
        qe4p = P(name="qe4p", bufs=1)   # qe4 [128,8,4,N]
        acc4p = P(name="acc4p", bufs=1)  # acc4 / jacc [128,8,4,N]
        actp = P(name="actp", bufs=3)   # sum_t,sum_s,joint,vj,qj [128,8,N]
        meanp = P(name="meanp", bufs=5)  # mts,mst,mtj,mjt,mjs
        accp = P(name="accp", bufs=2)   # cacc/f2/h2 [128,8,N]
        gatep = P(name="gatep", bufs=2)
        qkp = P(name="qkp", bufs=2)     # qk products, sq, av tmp [128,8,N]
        xbp = P(name="xbp", bufs=2)     # xt_b, xs_b (live whole pass)
        xinp = P(name="xinp", bufs=1)   # [128,4,N] f32 staging
        smp = P(name="smp", bufs=1)     # softmax exp [8,4,N] bf16
        smdp = P(name="smdp", bufs=2)   # softmax denom [8,N] f32
        smbp = P(name="smbp", bufs=4)   # a_l bf16 [8,N]
        smrp = P(name="smrp", bufs=4)   # LN scalars [1,N] f32
        smabp = P(name="smabp", bufs=4)  # abar f32 [8,N]

        _tc = [0]
        def T(pool, shape, dtype, tag):
            _tc[0] += 1
            return pool.tile(shape, dtype, tag=tag, name=f"{tag}_{_tc[0]}")

        ones_b = T(cst, [128, 1], BF16, "ones_b")
        nc.any.memset(ones_b[:], 1.0)
        onerow_f = T(cst, [1, 128], F32, "onerow_f")
        nc.any.memset(onerow_f[:], 1.0)
        sel = T(cst, [8, 8 * 128], BF16, "sel")
        nc.sync.dma_start(out=sel[:], in_=sel_d)
        o32 = T(cst, [128, 64], BF16, "o32")   # all-ones column at col 32
        nc.sync.dma_start(out=o32[:], in_=o32_d)
        i128 = T(cst, [128, 128], BF16, "i128")
        nc.sync.dma_start(out=i128[:], in_=i128_d)
        eps_t = T(cst, [1, 1], F32, "eps_t")
        nc.any.memset(eps_t[:], EPS)

        def ctile(name, ap):
            t = cst.tile(list(ap.shape), ap.dtype, tag=name)
            nc.sync.dma_start(out=t[:], in_=ap)
            return t

        bexp_t = [ctile(f"bexp{j}", bexp[j]) for j in range(2)]
        bqkv_t = [ctile(f"bqkv{i}", bqkv[i]) for i in range(5)]
        bout_t = [ctile(f"bout{i}", bout[i]) for i in range(5)]
        bjoint_t = ctile("bjoint", bjoint)
        bgate_t = [ctile(f"bgate{g}", bgate[g]) for g in range(3)]
        bo1_t = ctile("bo1", bo1)
        bo2_t = ctile("bo2", bo2)
        lng_t = [ctile(f"lng{i}", lng[i]) for i in range(3)]
        lnb_t = [ctile(f"lnb{i}", lnb[i]) for i in range(3)]

        def projS(w_d, M, src_pair, evict2, wcol0=0, npair=2, extra_mm=None):
            """Shared-weight projection, K=1024. src_pair(kt, p) -> [128,2,N]
            moving pair; two pairs (4 sources) per stationary load.
            evict2(mj, psums): psums[p] = [128,512] = pair p's two outputs.
            extra_mm(mj, p, psum): optional accumulate hook (residual)."""
            nmt = M // 128
            for mj0 in range(0, nmt, 2):
                gm = min(2, nmt - mj0)
                wt = T(wgp, [128, 8, 256], BF16, "wg")
                nc.sync.dma_start(
                    out=wt[:, :, : gm * 128],
                    in_=w_d[:, :, wcol0 + mj0 * 128 : wcol0 + (mj0 + gm) * 128],
                )
                for mj in range(mj0, mj0 + gm):
                    ps = [T(mmp, [128, 512], F32, "mm") for _ in range(npair)]
                    for kt in range(8):
                        w_sl = wt[:, kt, (mj - mj0) * 128 : (mj - mj0 + 1) * 128]
                        last = kt == 7 and extra_mm is None
                        for p in range(npair):
                            nc.tensor.matmul(ps[p][:], w_sl, src_pair(kt, p),
                                             start=(kt == 0), stop=last)
                    if extra_mm is not None:
                        for p in range(npair):
                            extra_mm(mj, p, ps[p])
                    evict2(mj, ps)

        def projM(w_d, M, K, src, evict2m, wcol0=0):
            """Single-source projection. One PSUM bank per m-tile (interleaved
            accumulation groups must not share a bank: start=True clears the
            has-written bits bank-wide). evict2m(mj0, gm, ps_list)."""
            nmt, nkt = M // 128, K // 128
            for mj0 in range(0, nmt, 2):
                gm = min(2, nmt - mj0)
                ps = [T(mmp, [128, 512], F32, "mm") for _ in range(gm)]
                for kc0 in range(0, nkt, 8):
                    kc = min(8, nkt - kc0)
                    wt = T(wgp, [128, 8, 256], BF16, "wg")
                    nc.sync.dma_start(
                        out=wt[:, :kc, : gm * 128],
                        in_=w_d[:, kc0 : kc0 + kc,
                                wcol0 + mj0 * 128 : wcol0 + (mj0 + gm) * 128],
                    )
                    for kt in range(kc0, kc0 + kc):
                        s = src(kt)
                        for mi in range(gm):
                            nc.tensor.matmul(
                                ps[mi][:, 0:256],
                                wt[:, kt - kc0, mi * 128 : (mi + 1) * 128], s,
                                start=(kt == 0), stop=(kt == nkt - 1))
                evict2m(mj0, gm, ps)

        def ev_split(dsts_of_mj, btile, bcol_of_mj, func=AF.Identity):
            """projM eviction: per-mtile ACT evicts [128,256] with bias."""
            def _ev(mj0, gm, ps):
                for mi in range(gm):
                    nc.scalar.activation(
                        dsts_of_mj(mj0 + mi), ps[mi][:, 0:256],
                        func, bias=btile[:, bcol_of_mj(mj0 + mi)
                                         : bcol_of_mj(mj0 + mi) + 1])
            return _ev

        def scores_all(q_sl, k4):
            """psum [8,4,N]: row h of col-block e2 = q[h].k[e2,h] (q pre-scaled)."""
            sp = T(scp, [8, 4, N], F32, "sc")
            for e2 in range(4):
                p = T(qkp, [128, 8, N], BF16, "qk")
                nc.vector.tensor_tensor(
                    out=p[:], in0=q_sl, in1=k4[:, :, e2, :], op=ALU.mult)
                for kt in range(8):
                    nc.tensor.matmul(sp[:, e2, :], o32[:, 32 - kt : 40 - kt],
                                     p[:, kt, :], start=(kt == 0), stop=(kt == 7))
            return sp

        def softmax_tiles(sp):
            """sp [8,4,N] psum scores -> 4 bf16 [8,N] attention-weight tiles."""
            et = T(smp, [8, 4, N], BF16, "sm")
            nc.scalar.activation(et[:], sp[:], AF.Exp)
            d = T(smdp, [8, N], F32, "smd")
            nc.vector.tensor_add(out=d[:], in0=et[:, 0, :], in1=et[:, 1, :])
            for e2 in (2, 3):
                nc.vector.tensor_add(out=d[:], in0=d[:], in1=et[:, e2, :])
            r = T(smdp, [8, N], F32, "smd")
            nc.vector.reciprocal(r[:], d[:])
            outs = []
            for e2 in range(4):
                a = T(smbp, [8, N], BF16, "smb")
                nc.vector.tensor_tensor(out=a[:], in0=et[:, e2, :], in1=r[:],
                                        op=ALU.mult)
                outs.append(a)
            return outs

        def av_accum(a_list, v4, dst_sl):
            """dst_sl [128,8,N] = sum_e2 broadcast_heads(a_list[e2]) * V[e2]."""
            for e2 in range(4):
                cur = dst_sl if e2 == 0 else T(qkp, [128, 8, N], BF16, "qk")
                for mt in range(8):
                    bp = T(brp, [128, N], F32, "br")
                    nc.tensor.matmul(bp[:], sel[:, mt * 128 : (mt + 1) * 128],
                                     a_list[e2][:], start=True, stop=True)
                    nc.vector.tensor_tensor(
                        out=cur[:, mt, :], in0=bp[:],
                        in1=v4[:, mt, e2, :], op=ALU.mult)
                if e2 > 0:
                    nc.vector.tensor_add(out=dst_sl, in0=dst_sl, in1=cur[:])

        def ln_norm(x_sl, g_t, b_t, dst_of_mt):
            """LayerNorm over the 1024 feats of x_sl [128,8,N] (bf16, in-place
            scratch); writes normalized*g+b to dst_of_mt(mt)."""
            sq = T(qkp, [128, 8, N], BF16, "qk")
            nc.vector.tensor_tensor(out=sq[:], in0=x_sl, in1=x_sl, op=ALU.mult)
            st_s = T(scp, [1, N], F32, "sc")
            for kt in range(8):
                nc.tensor.matmul(st_s[:], ones_b[:], x_sl[:, kt, :],
                                 start=(kt == 0), stop=(kt == 7))
            st_q = T(scp, [1, N], F32, "sc")
            for kt in range(8):
                nc.tensor.matmul(st_q[:], ones_b[:], sq[:, kt, :],
                                 start=(kt == 0), stop=(kt == 7))
            mean = T(smrp, [1, N], F32, "smr")
            nc.vector.tensor_scalar_mul(mean[:], st_s[:], 1.0 / HD)
            msq = T(smrp, [1, N], F32, "smr")
            nc.vector.tensor_scalar_mul(msq[:], st_q[:], 1.0 / HD)
            var = T(smrp, [1, N], F32, "smr")
            nc.vector.tensor_tensor(out=var[:], in0=mean[:], in1=mean[:],
                                    op=ALU.mult)
            nc.vector.tensor_tensor(out=var[:], in0=msq[:], in1=var[:],
                                    op=ALU.subtract)
            std = T(smrp, [1, N], F32, "smr")
            nc.scalar.activation(std[:], var[:], AF.Sqrt, bias=eps_t[:])
            rstd = T(smrp, [1, N], F32, "smr")
            nc.vector.reciprocal(rstd[:], std[:])
            mb = T(brp, [128, N], F32, "br")
            nc.tensor.matmul(mb[:], onerow_f[:], mean[:], start=True, stop=True)
            rb = T(brp, [128, N], F32, "br")
            nc.tensor.matmul(rb[:], onerow_f[:], rstd[:], start=True, stop=True)
            for mt in range(8):
                nc.vector.tensor_tensor(out=x_sl[:, mt, :], in0=x_sl[:, mt, :],
                                        in1=mb[:], op=ALU.subtract)
                nc.vector.tensor_tensor(out=x_sl[:, mt, :], in0=x_sl[:, mt, :],
                                        in1=rb[:], op=ALU.mult)
                nc.vector.tensor_scalar(
                    out=dst_of_mt(mt), in0=x_sl[:, mt, :],
                    scalar1=g_t[:, mt : mt + 1], scalar2=b_t[:, mt : mt + 1],
                    op0=ALU.mult, op1=ALU.add)

        def kv_proj(mi, src4):
            """K/V projection of mha mi from src4 [128,8,4,N] -> k4, v4."""
            k4 = T(qkvp, [128, 8, 4, N], BF16, "qkv")
            v4 = T(qkvp, [128, 8, 4, N], BF16, "qkv")
            def ev(mj, ps):
                dst = k4 if mj < 8 else v4
                bcol = 8 + mj      # k tiles: cols 8..15, v tiles: 16..23
                for p in range(2):
                    nc.scalar.activation(
                        dst[:, mj % 8, 2 * p : 2 * p + 2, :], ps[p][:],
                        AF.Identity,
                        bias=bqkv_t[mi][:, bcol : bcol + 1])
            projS(wqkv[mi], 2 * HD,
                  lambda kt, p: src4[:, kt, 2 * p : 2 * p + 2, :], ev,
                  wcol0=HD)
            return k4, v4

        def q_proj(mi, src4):
            qe4 = T(qe4p, [128, 8, 4, N], BF16, "qe4")
            def ev(mj, ps):
                for p in range(2):
                    nc.scalar.activation(
                        qe4[:, mj, 2 * p : 2 * p + 2, :], ps[p][:],
                        AF.Identity, bias=bqkv_t[mi][:, mj : mj + 1])
            projS(wqkv[mi], HD,
                  lambda kt, p: src4[:, kt, 2 * p : 2 * p + 2, :], ev)
            return qe4

        for c in range(NP):
            bs = slice(c * N, (c + 1) * N)

            def load_x(xd):
                xb = T(xbp, [128, 8, N], BF16, "xb")
                for h in range(2):
                    xf = T(xinp, [128, 4, N], F32, "xin")
                    nc.sync.dma_start(out=xf[:], in_=xd[:, 4 * h : 4 * h + 4, bs])
                    nc.vector.tensor_copy(out=xb[:, 4 * h : 4 * h + 4, :],
                                          in_=xf[:])
                return xb

            xt_b = load_x(xt_d)
            xs_b = load_x(xs_d)

            def run_self(x_b, j, sum_dst, enh_dst):
                """expand -> self-MHA(j) -> +resid -> LN -> enh_dst [128,8,4,N];
                sum_dst [128,8,N] = sum_e enh."""
                xp = T(expp, [128, 8, 4, N], BF16, "exp")
                # expand: m-tile m = e*8+mj -> xp[:, mj, e, :]
                def ev_exp(mj0, gm, ps):
                    for mi in range(gm):
                        m = mj0 + mi
                        nc.scalar.activation(
                            xp[:, m % 8, m // 8, :], ps[mi][:, 0:256],
                            AF.Identity, bias=bexp_t[j][:, m : m + 1])
                projM(wexp[j], E * HD, HD, lambda kt: x_b[:, kt, :], ev_exp)

                k4, v4 = kv_proj(j, xp)
                qe4 = q_proj(j, xp)
                acc4 = T(acc4p, [128, 8, 4, N], BF16, "acc4")
                for e1 in range(4):
                    a_l = softmax_tiles(scores_all(qe4[:, :, e1, :], k4))
                    av_accum(a_l, v4, acc4[:, :, e1, :])
                # out proj with residual folded into PSUM; evict in place to xp
                def ex_res(mj, p, ps):
                    nc.tensor.matmul(ps[:], i128[:],
                                     xp[:, mj, 2 * p : 2 * p + 2, :],
                                     start=False, stop=True)
                def ev_out(mj, ps):
                    for p in range(2):
                        nc.scalar.activation(
                            xp[:, mj, 2 * p : 2 * p + 2, :], ps[p][:],
                            AF.Identity, bias=bout_t[j][:, mj : mj + 1])
                projS(wout[j], HD,
                      lambda kt, p: acc4[:, kt, 2 * p : 2 * p + 2, :],
                      ev_out, extra_mm=ex_res)
                for e1 in range(4):
                    ln_norm(xp[:, :, e1, :], lng_t[j], lnb_t[j],
                            lambda mt, e1=e1: enh_dst[:, mt, e1, :])
                t2 = T(qkp, [128, 8, N], BF16, "qk")
                nc.vector.tensor_add(out=sum_dst[:], in0=enh_dst[:, :, 0, :],
                                     in1=enh_dst[:, :, 1, :])
                nc.vector.tensor_add(out=t2[:], in0=enh_dst[:, :, 2, :],
                                     in1=enh_dst[:, :, 3, :])
                nc.vector.tensor_add(out=sum_dst[:], in0=sum_dst[:], in1=t2[:])

            t_enh = T(enhp, [128, 8, 4, N], BF16, "enh")
            s_enh = T(enhp, [128, 8, 4, N], BF16, "enh")
            sum_t = T(actp, [128, 8, N], BF16, "act")
            sum_s = T(actp, [128, 8, N], BF16, "act")
            run_self(xt_b, 0, sum_t, t_enh)
            run_self(xs_b, 1, sum_s, s_enh)

            joint = T(actp, [128, 8, N], BF16, "act")
            projM(wjoint, HD, 2 * HD,
                  lambda kt: sum_t[:, kt, :] if kt < 8 else sum_s[:, kt - 8, :],
                  ev_split(lambda mj: joint[:, mj, :], bjoint_t, lambda mj: mj))

            def run_cross(mi, qsrc4, kvsrc4, dst):
                """cross-attn, output mean over query positions -> dst [128,8,N].
                wout[mi] pre-scaled by 0.25 on host."""
                k4, v4 = kv_proj(mi, kvsrc4)
                qe4 = q_proj(mi, qsrc4)
                abar = [None] * 4
                for e1 in range(4):
                    a_l = softmax_tiles(scores_all(qe4[:, :, e1, :], k4))
                    for e2 in range(4):
                        if e1 == 0:
                            ab = T(smabp, [8, N], F32, "smab")
                            nc.vector.tensor_copy(out=ab[:], in_=a_l[e2][:])
                            abar[e2] = ab
                        else:
                            nc.vector.tensor_add(out=abar[e2][:], in0=abar[e2][:],
                                                 in1=a_l[e2][:])
                abar_b = []
                for e2 in range(4):
                    ab = T(smbp, [8, N], BF16, "smb")
                    nc.vector.tensor_copy(out=ab[:], in_=abar[e2][:])
                    abar_b.append(ab)
                cacc = T(accp, [128, 8, N], BF16, "accx")
                av_accum(abar_b, v4, cacc[:])
                projM(wout[mi], HD, HD, lambda kt: cacc[:, kt, :],
                      ev_split(lambda mj: dst[:, mj, :], bout_t[mi],
                               lambda mj: mj))

            mts = T(meanp, [128, 8, N], BF16, "mean")
            mst = T(meanp, [128, 8, N], BF16, "mean")
            run_cross(2, t_enh, s_enh, mts)
            run_cross(3, s_enh, t_enh, mst)

            # mha4 group: mtj == msj == Wout4 @ (Wv4 @ joint + bv4) + bout4
            vj = T(actp, [128, 8, N], BF16, "act")
            projM(wqkv[4], HD, HD, lambda kt: joint[:, kt, :],
                  ev_split(lambda mj: vj[:, mj, :], bqkv_t[4],
                           lambda mj: 16 + mj), wcol0=2 * HD)
            mtj = T(meanp, [128, 8, N], BF16, "mean")
            projM(wout[4], HD, HD, lambda kt: vj[:, kt, :],
                  ev_split(lambda mj: mtj[:, mj, :], bout_t[4], lambda mj: mj))
            qj = T(actp, [128, 8, N], BF16, "act")
            projM(wqkv[4], HD, HD, lambda kt: joint[:, kt, :],
                  ev_split(lambda mj: qj[:, mj, :], bqkv_t[4], lambda mj: mj))

            # jx: single-query cross-attn (q = joint row), kv from t_enh/s_enh
            jacc = T(acc4p, [128, 8, 4, N], BF16, "acc4")
            for jj, enh in enumerate((t_enh, s_enh)):
                k4, v4 = kv_proj(4, enh)
                a_l = softmax_tiles(scores_all(qj[:], k4))
                av_accum(a_l, v4, jacc[:, :, jj, :])
            mjt = T(meanp, [128, 8, N], BF16, "mean")
            mjs = T(meanp, [128, 8, N], BF16, "mean")
            def ev_jx(mj, ps):
                for jj, dst in enumerate((mjt, mjs)):
                    nc.scalar.activation(
                        dst[:, mj, :], ps[0][:, jj * 256 : (jj + 1) * 256],
                        AF.Identity, bias=bout_t[4][:, mj : mj + 1])
            projS(wout[4], HD, lambda kt, p: jacc[:, kt, 0:2, :], ev_jx,
                  npair=1)

            # gates (sigmoid) + fused mults, interleaved to bound liveness
            def run_gate(g, in_a, in_b):
                gt = T(gatep, [128, 8, N], BF16, "gate")
                projM(wgate[g], HD, 2 * HD,
                      lambda kt: in_a[:, kt, :] if kt < 8 else in_b[:, kt - 8, :],
                      ev_split(lambda mj: gt[:, mj, :], bgate_t[g],
                               lambda mj: mj, func=AF.Sigmoid))
                return gt

            f2 = T(accp, [128, 8, N], BF16, "accx")
            gate_t = run_gate(0, mts, mtj)
            nc.vector.tensor_tensor(out=f2[:], in0=gate_t[:], in1=mtj[:],
                                    op=ALU.mult)
            nc.vector.tensor_tensor(out=mts[:], in0=gate_t[:], in1=mts[:],
                                    op=ALU.mult)
            gate_s = run_gate(1, mst, mtj)
            nc.vector.tensor_tensor(out=mst[:], in0=gate_s[:], in1=mst[:],
                                    op=ALU.mult)
            nc.vector.tensor_tensor(out=mtj[:], in0=gate_s[:], in1=mtj[:],
                                    op=ALU.mult)
            gate_j = run_gate(2, mjt, mjs)
            nc.vector.tensor_tensor(out=mjt[:], in0=gate_j[:], in1=mjt[:],
                                    op=ALU.mult)
            nc.vector.tensor_tensor(out=mjs[:], in0=gate_j[:], in1=mjs[:],
                                    op=ALU.mult)
            fs = [mts, mst, f2, mtj, mjt, mjs]

            h1 = T(expp, [128, 8, 4, N], BF16, "exp")
            def ev_h1(mj0, gm, ps):
                for mi in range(gm):
                    m = mj0 + mi
                    nc.scalar.activation(
                        h1[:, m % 8, m // 8, :], ps[mi][:, 0:256],
                        AF.Relu, bias=bo1_t[:, m : m + 1])
            projM(wo1, 2 * HD, 6 * HD, lambda kt: fs[kt // 8][:, kt % 8, :],
                  ev_h1)
            h2 = T(accp, [128, 8, N], BF16, "accx")
            projM(wo2, HD, 2 * HD, lambda kt: h1[:, kt % 8, kt // 8, :],
                  ev_split(lambda mj: h2[:, mj, :], bo2_t, lambda mj: mj))

            # final LN (g,b pre-scaled by res_w) + (1-res_w)/2*(xt+xs)
            yt = T(qkp, [128, 8, N], BF16, "qk")
            ln_norm(h2[:], lng_t[2], lnb_t[2], lambda mt: yt[:, mt, :])
            cres = (1.0 - res_w) * 0.5
            orig = T(qkp, [128, 8, N], BF16, "qk")
            nc.vector.tensor_add(out=orig[:], in0=xt_b[:], in1=xs_b[:])
            nc.vector.tensor_scalar_mul(orig[:], orig[:], cres)
            nc.vector.tensor_add(out=yt[:], in0=yt[:], in1=orig[:])
            nc.sync.dma_start(out=y_d[:, :, bs], in_=yt[:])

    nc.compile()
    return nc


def _sel_const():
    s = np.zeros((8, 8 * 128), np.float32)
    for mt in range(8):
        s[mt, mt * 128 : (mt + 1) * 128] = 1.0
    return s.astype(BF)


def _o32_const():
    o = np.zeros((128, 64), np.float32)
    o[:, 32] = 1.0
    return o.astype(BF)


def _wl(w):
    """torch-style [M_out, K_in] -> [128, K/128, M] bf16 (p = row in k-tile)."""
    a = np.asarray(w, np.float32).T          # [K, M]
    K, M = a.shape
    a = a.reshape(K // 128, 128, M).transpose(1, 0, 2)
    return np.ascontiguousarray(a).astype(BF)


def _prep_inputs(i):
    res_w = float(np.asarray(i["res_w"]).reshape(-1)[0])
    sc = 1.0 / math.sqrt(128.0)

    shared = {
        "wexp0": _wl(i["exp_t_w"]), "wexp1": _wl(i["exp_s_w"]),
        "bexp0": _bias_cols(np.asarray(i["exp_t_b"]) + np.asarray(i["pos_enc"]).reshape(-1)),
        "bexp1": _bias_cols(np.asarray(i["exp_s_b"]) + np.asarray(i["pos_enc"]).reshape(-1)),
        "wjoint": _wl(np.asarray(i["joint_w"], np.float32) * 0.25),
        "bjoint": _bias_cols(i["joint_b"]),
        "wo1": _wl(i["out1_w"]), "bo1": _bias_cols(i["out1_b"]),
        "wo2": _wl(i["out2_w"]), "bo2": _bias_cols(i["out2_b"]),
        "sel_c": _sel_const(), "o32_c": _o32_const(),
        "i128_c": np.eye(128, dtype=np.float32).astype(BF),
    }
    for g in range(3):
        shared[f"wgate{g}"] = _wl(i["gate_w"][g])
        shared[f"bgate{g}"] = _bias_cols(i["gate_b"][g])
    for m in range(5):
        w = np.asarray(i["mha_in_w"][m], np.float32).copy()
        b = np.asarray(i["mha_in_b"][m], np.float32).copy()
        w[:HD] *= sc
        b[:HD] *= sc
        shared[f"wqkv{m}"] = _wl(w)
        shared[f"bqkv{m}"] = _bias_cols(b)
        wo = np.asarray(i["mha_out_w"][m], np.float32)
        if m in (2, 3):
            wo = wo * 0.25      # fold mean over the 4 query positions
        shared[f"wout{m}"] = _wl(wo)
        shared[f"bout{m}"] = _bias_cols(i["mha_out_b"][m])
    for ln in range(3):
        g = np.asarray(i["ln_g"][ln], np.float32)
        b = np.asarray(i["ln_b"][ln], np.float32)
        if ln == 2:
            g = g * res_w
            b = b * res_w
        shared[f"lng{ln}"] = _bias_cols(g)
        shared[f"lnb{ln}"] = _bias_cols(b)

    def shard_x(x, c):
        xc = np.asarray(x, np.float32)[c * BC : (c + 1) * BC, 0, :]  # [512,1024]
        return np.ascontiguousarray(xc.T.reshape(8, 128, BC).transpose(1, 0, 2))

    in_maps = []
    for c in range(NCORES):
        m = dict(shared)
        m["xt"] = shard_x(i["temporal_features"], c)
        m["xs"] = shard_x(i["spatial_features"], c)
        in_maps.append(m)
    return res_w, in_maps


def kernel(**inputs):
    res_w, in_maps = _prep_inputs(inputs)
    nc = build(res_w)
    res = bass_utils.run_bass_kernel_spmd(nc, in_maps, core_ids=list(range(NCORES)))
    outs = []
    for c in range(NCORES):
        y = res.results[c]["y"]                                   # [128,8,512]
        outs.append(np.asarray(y).astype(np.float32)
                    .transpose(1, 0, 2).reshape(HD, BC).T)
    return np.concatenate(outs, 0)[:, None, :].astype(np.float32)


# revision 15
# speedup vs baseline: 1.7529x; 1.0347x over previous
"""Trainium2 Bass kernel for nn_MISA (dense_transformer, data-parallel over 8 cores).

Layout: feature-major activations [feat_part=128, mtile, batch_cols] per core.
Batch 4096 -> 512 per core -> two passes of 256 columns.
All matmuls bf16 (fp32 PSUM accumulation); LN/softmax internals fp32.

v2 (DMA restructure): weights live in DRAM as [128, K/128, M] and each
weight tile is DMA'd exactly once per pass:
- projS shares one stationary weight load across the 4 expand positions
  (pairs of positions ride one matmul: moving [128,2,N] -> PSUM [128,512]).
- PSUM evictions run on the Scalar engine (Identity+bias) to unload DVE.
- self-attn residual is folded into the out-proj PSUM via an identity matmul,
  and the out-proj evicts in place over the expand tile.
- output is written bf16 and cast to f32 on host.

Structural simplifications (exact, not approximations):
- attention with all-equal keys/values (q/k/v = broadcast joint row) is the
  identity on v: cross_tj == cross_sj == out_proj4(v_proj4(joint)).
- mean over query positions commutes with out_proj and with A@V, so the six
  cross outputs never materialize per-query outputs (abar-weighted V only).
- all-equal queries (j as q): single query row, output equals its mean.
"""
import sys, math
from contextlib import ExitStack
sys.path.insert(0, "/opt/trn_rl_repo")

import numpy as np
import ml_dtypes

import concourse.bass as bass
import concourse.mybir as mybir
from concourse import bacc
import concourse.tile as tile
from concourse import bass_utils

F32 = mybir.dt.float32
BF16 = mybir.dt.bfloat16
AF = mybir.ActivationFunctionType
ALU = mybir.AluOpType
BF = ml_dtypes.bfloat16

H = 8
E = 4
HD = 1024
B = 4096
NCORES = 8
BC = B // NCORES          # 512 batch per core
NP = 2                    # passes per core
N = BC // NP              # 256 batch cols per pass
EPS = 1e-5


def _bias_cols(b):
    # [M] -> [128, M//128]: column m = per-partition bias of m-tile m
    return np.ascontiguousarray(np.asarray(b, np.float32).reshape(-1, 128).T)


def build(res_w: float):
    nc = bacc.Bacc("TRN2", target_bir_lowering=False, debug=False)

    def din(name, shape, dt):
        return nc.dram_tensor(name, list(shape), dt, kind="ExternalInput").ap()

    xt_d = din("xt", (128, 8, BC), F32)
    xs_d = din("xs", (128, 8, BC), F32)
    # weights as [128, K/128, M] (partition = row within k-tile)
    wexp = [din(f"wexp{j}", (128, 8, E * HD), BF16) for j in range(2)]
    bexp = [din(f"bexp{j}", (128, 32), F32) for j in range(2)]
    wqkv = [din(f"wqkv{i}", (128, 8, 3 * HD), BF16) for i in range(5)]
    bqkv = [din(f"bqkv{i}", (128, 24), F32) for i in range(5)]
    wout = [din(f"wout{i}", (128, 8, HD), BF16) for i in range(5)]
    bout = [din(f"bout{i}", (128, 8), F32) for i in range(5)]
    wjoint = din("wjoint", (128, 16, HD), BF16)
    bjoint = din("bjoint", (128, 8), F32)
    wgate = [din(f"wgate{g}", (128, 16, HD), BF16) for g in range(3)]
    bgate = [din(f"bgate{g}", (128, 8), F32) for g in range(3)]
    wo1 = din("wo1", (128, 48, 2 * HD), BF16)
    bo1 = din("bo1", (128, 16), F32)
    wo2 = din("wo2", (128, 16, HD), BF16)
    bo2 = din("bo2", (128, 8), F32)
    lng = [din(f"lng{i}", (128, 8), F32) for i in range(3)]
    lnb = [din(f"lnb{i}", (128, 8), F32) for i in range(3)]
    sel_d = din("sel_c", (8, 8 * 128), BF16)
    o32_d = din("o32_c", (128, 64), BF16)
    i128_d = din("i128_c", (128, 128), BF16)
    y_d = nc.dram_tensor("y", [128, 8, BC], BF16, kind="ExternalOutput").ap()

    with tile.TileContext(nc) as tc, ExitStack() as ctx:
        P = lambda **kw: ctx.enter_context(tc.tile_pool(**kw))
        cst = P(name="cst", bufs=1)
        wgp = P(name="wgp", bufs=2)                 # [128,8,256] weight blocks
        mmp = P(name="mmp", bufs=4, space="PSUM")   # 4 x [128,512] banks
        scp = P(name="scp", bufs=1, space="PSUM")   # [8,4,N] = 2 banks
        brp = P(name="brp", bufs=2, space="PSUM")   # 2 x [128,N] banks
        expp = P(name="expp", bufs=1)   # xp [128,8,4,N]; also h1
        enhp = P(name="enhp", bufs=2)   # t_enh, s_enh [128,8,4,N]
        qkvp = P(name="qkvp", bufs=2)   # k4, v4 [128,8,4,N]
        qe4p = P(name="qe4p", bufs=1)   # qe4 [128,8,4,N]
        acc4p = P(name="acc4p", bufs=1)  # acc4 / jacc [128,8,4,N]
        actp = P(name="actp", bufs=3)   # sum_t,sum_s,joint,vj,qj [128,8,N]
        meanp = P(name="meanp", bufs=5)  # mts,mst,mtj,mjt,mjs
        accp = P(name="accp", bufs=2)   # cacc/f2/h2 [128,8,N]
        gatep = P(name="gatep", bufs=2)
        qkp = P(name="qkp", bufs=2)     # qk products, sq, av tmp [128,8,N]
        xbp = P(name="xbp", bufs=2)     # xt_b, xs_b (live whole pass)
        xinp = P(name="xinp", bufs=1)   # [128,4,N] f32 staging
        smp = P(name="smp", bufs=1)     # softmax exp [8,4,N] bf16
        smdp = P(name="smdp", bufs=2)   # softmax denom [8,N] f32
        smbp = P(name="smbp", bufs=4)   # a_l bf16 [8,N]
        smrp = P(name="smrp", bufs=3)   # LN scalars [1,N] f32
        smabp = P(name="smabp", bufs=4)  # abar f32 [8,N]

        _tc = [0]
        def T(pool, shape, dtype, tag):
            _tc[0] += 1
            return pool.tile(shape, dtype, tag=tag, name=f"{tag}_{_tc[0]}")

        ones_b = T(cst, [128, 1], BF16, "ones_b")
        nc.any.memset(ones_b[:], 1.0)
        onerow_f = T(cst, [1, 128], F32, "onerow_f")
        nc.any.memset(onerow_f[:], 1.0)
        sel = T(cst, [8, 8 * 128], BF16, "sel")
        nc.sync.dma_start(out=sel[:], in_=sel_d)
        o32 = T(cst, [128, 64], BF16, "o32")   # all-ones column at col 32
        nc.sync.dma_start(out=o32[:], in_=o32_d)
        i128 = T(cst, [128, 128], BF16, "i128")
        nc.sync.dma_start(out=i128[:], in_=i128_d)
        eps_t = T(cst, [1, 1], F32, "eps_t")
        nc.any.memset(eps_t[:], EPS)

        def ctile(name, ap):
            t = cst.tile(list(ap.shape), ap.dtype, tag=name)
            nc.sync.dma_start(out=t[:], in_=ap)
            return t

        bexp_t = [ctile(f"bexp{j}", bexp[j]) for j in range(2)]
        bqkv_t = [ctile(f"bqkv{i}", bqkv[i]) for i in range(5)]
        bout_t = [ctile(f"bout{i}", bout[i]) for i in range(5)]
        bjoint_t = ctile("bjoint", bjoint)
        bgate_t = [ctile(f"bgate{g}", bgate[g]) for g in range(3)]
        bo1_t = ctile("bo1", bo1)
        bo2_t = ctile("bo2", bo2)
        lng_t = [ctile(f"lng{i}", lng[i]) for i in range(3)]
        lnb_t = [ctile(f"lnb{i}", lnb[i]) for i in range(3)]

        def projS(w_d, M, src_pair, evict2, wcol0=0, npair=2, extra_mm=None):
            """Shared-weight projection, K=1024. src_pair(kt, p) -> [128,2,N]
            moving pair; two pairs (4 sources) per stationary load.
            evict2(mj, psums): psums[p] = [128,512] = pair p's two outputs.
            extra_mm(mj, p, psum): optional accumulate hook (residual)."""
            nmt = M // 128
            for mj0 in range(0, nmt, 2):
                gm = min(2, nmt - mj0)
                wt = T(wgp, [128, 8, 256], BF16, "wg")
                nc.sync.dma_start(
                    out=wt[:, :, : gm * 128],
                    in_=w_d[:, :, wcol0 + mj0 * 128 : wcol0 + (mj0 + gm) * 128],
                )
                for mj in range(mj0, mj0 + gm):
                    ps = [T(mmp, [128, 512], F32, "mm") for _ in range(npair)]
                    for kt in range(8):
                        w_sl = wt[:, kt, (mj - mj0) * 128 : (mj - mj0 + 1) * 128]
                        last = kt == 7 and extra_mm is None
                        for p in range(npair):
                            nc.tensor.matmul(ps[p][:], w_sl, src_pair(kt, p),
                                             start=(kt == 0), stop=last)
                    if extra_mm is not None:
                        for p in range(npair):
                            extra_mm(mj, p, ps[p])
                    evict2(mj, ps)

        def projM(w_d, M, K, src, evict2m, wcol0=0):
            """Single-source projection. One PSUM bank per m-tile (interleaved
            accumulation groups must not share a bank: start=True clears the
            has-written bits bank-wide). evict2m(mj0, gm, ps_list)."""
            nmt, nkt = M // 128, K // 128
            for mj0 in range(0, nmt, 2):
                gm = min(2, nmt - mj0)
                ps = [T(mmp, [128, 512], F32, "mm") for _ in range(gm)]
                for kc0 in range(0, nkt, 8):
                    kc = min(8, nkt - kc0)
                    wt = T(wgp, [128, 8, 256], BF16, "wg")
                    nc.sync.dma_start(
                        out=wt[:, :kc, : gm * 128],
                        in_=w_d[:, kc0 : kc0 + kc,
                                wcol0 + mj0 * 128 : wcol0 + (mj0 + gm) * 128],
                    )
                    for kt in range(kc0, kc0 + kc):
                        s = src(kt)
                        for mi in range(gm):
                            nc.tensor.matmul(
                                ps[mi][:, 0:256],
                                wt[:, kt - kc0, mi * 128 : (mi + 1) * 128], s,
                                start=(kt == 0), stop=(kt == nkt - 1))
                evict2m(mj0, gm, ps)

        def ev_split(dsts_of_mj, btile, bcol_of_mj, func=AF.Identity):
            """projM eviction: per-mtile ACT evicts [128,256] with bias."""
            def _ev(mj0, gm, ps):
                for mi in range(gm):
                    nc.scalar.activation(
                        dsts_of_mj(mj0 + mi), ps[mi][:, 0:256],
                        func, bias=btile[:, bcol_of_mj(mj0 + mi)
                                         : bcol_of_mj(mj0 + mi) + 1])
            return _ev

        def scores_all(q_sl, k4):
            """psum [8,4,N]: row h of col-block e2 = q[h].k[e2,h] (q pre-scaled)."""
            sp = T(scp, [8, 4, N], F32, "sc")
            for e2 in range(4):
                p = T(qkp, [128, 8, N], BF16, "qk")
                nc.vector.tensor_tensor(
                    out=p[:], in0=q_sl, in1=k4[:, :, e2, :], op=ALU.mult)
                for kt in range(8):
                    nc.tensor.matmul(sp[:, e2, :], o32[:, 32 - kt : 40 - kt],
                                     p[:, kt, :], start=(kt == 0), stop=(kt == 7))
            return sp

        def softmax_tiles(sp):
            """sp [8,4,N] psum scores -> 4 bf16 [8,N] attention-weight tiles."""
            et = T(smp, [8, 4, N], BF16, "sm")
            nc.scalar.activation(et[:], sp[:], AF.Exp)
            d = T(smdp, [8, N], F32, "smd")
            nc.vector.tensor_add(out=d[:], in0=et[:, 0, :], in1=et[:, 1, :])
            for e2 in (2, 3):
                nc.vector.tensor_add(out=d[:], in0=d[:], in1=et[:, e2, :])
            r = T(smdp, [8, N], F32, "smd")
            nc.vector.reciprocal_approx_fast(out=r[:], in_=d[:])
            outs = []
            for e2 in range(4):
                a = T(smbp, [8, N], BF16, "smb")
                nc.vector.tensor_tensor(out=a[:], in0=et[:, e2, :], in1=r[:],
                                        op=ALU.mult)
                outs.append(a)
            return outs

        def av_accum(a_list, v4, dst_sl):
            """dst_sl [128,8,N] = sum_e2 broadcast_heads(a_list[e2]) * V[e2]."""
            for e2 in range(4):
                cur = dst_sl if e2 == 0 else T(qkp, [128, 8, N], BF16, "qk")
                for mt in range(8):
                    bp = T(brp, [128, N], F32, "br")
                    nc.tensor.matmul(bp[:], sel[:, mt * 128 : (mt + 1) * 128],
                                     a_list[e2][:], start=True, stop=True)
                    nc.vector.tensor_tensor(
                        out=cur[:, mt, :], in0=bp[:],
                        in1=v4[:, mt, e2, :], op=ALU.mult)
                if e2 > 0:
                    nc.vector.tensor_add(out=dst_sl, in0=dst_sl, in1=cur[:])

        def ln_norm(x_sl, g_t, b_t, dst_of_mt):
            """LayerNorm over the 1024 feats of x_sl [128,8,N] (bf16, in-place
            scratch); writes normalized*g+b to dst_of_mt(mt)."""
            sq = T(qkp, [128, 8, N], BF16, "qk")
            nc.vector.tensor_tensor(out=sq[:], in0=x_sl, in1=x_sl, op=ALU.mult)
            st_s = T(scp, [1, N], F32, "sc")
            for kt in range(8):
                nc.tensor.matmul(st_s[:], ones_b[:], x_sl[:, kt, :],
                                 start=(kt == 0), stop=(kt == 7))
            st_q = T(scp, [1, N], F32, "sc")
            for kt in range(8):
                nc.tensor.matmul(st_q[:], ones_b[:], sq[:, kt, :],
                                 start=(kt == 0), stop=(kt == 7))
            mean = T(smrp, [1, N], F32, "smr")
            nc.vector.tensor_scalar_mul(mean[:], st_s[:], 1.0 / HD)
            mb = T(brp, [128, N], F32, "br")
            nc.tensor.matmul(mb[:], onerow_f[:], mean[:], start=True, stop=True)
            msq = T(smrp, [1, N], F32, "smr")
            nc.vector.tensor_scalar_mul(msq[:], st_q[:], 1.0 / HD)
            var = T(smrp, [1, N], F32, "smr")
            nc.vector.tensor_tensor(out=var[:], in0=mean[:], in1=mean[:],
                                    op=ALU.mult)
            nc.vector.tensor_tensor(out=var[:], in0=msq[:], in1=var[:],
                                    op=ALU.subtract)
            std = T(smrp, [1, N], F32, "smr")
            nc.scalar.activation(std[:], var[:], AF.Sqrt, bias=eps_t[:])
            rstd = T(smrp, [1, N], F32, "smr")
            nc.vector.reciprocal_approx_fast(out=rstd[:], in_=std[:])
            rb = T(brp, [128, N], F32, "br")
            nc.tensor.matmul(rb[:], onerow_f[:], rstd[:], start=True, stop=True)
            for mt in range(8):
                nc.vector.tensor_tensor(out=x_sl[:, mt, :], in0=x_sl[:, mt, :],
                                        in1=mb[:], op=ALU.subtract)
                nc.vector.tensor_tensor(out=x_sl[:, mt, :], in0=x_sl[:, mt, :],
                                        in1=rb[:], op=ALU.mult)
                nc.vector.tensor_scalar(
                    out=dst_of_mt(mt), in0=x_sl[:, mt, :],
                    scalar1=g_t[:, mt : mt + 1], scalar2=b_t[:, mt : mt + 1],
                    op0=ALU.mult, op1=ALU.add)

        def kv_proj(mi, src4):
            """K/V projection of mha mi from src4 [128,8,4,N] -> k4, v4."""
            k4 = T(qkvp, [128, 8, 4, N], BF16, "qkv")
            v4 = T(qkvp, [128, 8, 4, N], BF16, "qkv")
            def ev(mj, ps):
                dst = k4 if mj < 8 else v4
                bcol = 8 + mj      # k tiles: cols 8..15, v tiles: 16..23
                for p in range(2):
                    nc.scalar.activation(
                        dst[:, mj % 8, 2 * p : 2 * p + 2, :], ps[p][:],
                        AF.Identity,
                        bias=bqkv_t[mi][:, bcol : bcol + 1])
            projS(wqkv[mi], 2 * HD,
                  lambda kt, p: src4[:, kt, 2 * p : 2 * p + 2, :], ev,
                  wcol0=HD)
            return k4, v4

        def q_proj(mi, src4):
            qe4 = T(qe4p, [128, 8, 4, N], BF16, "qe4")
            def ev(mj, ps):
                for p in range(2):
                    nc.scalar.activation(
                        qe4[:, mj, 2 * p : 2 * p + 2, :], ps[p][:],
                        AF.Identity, bias=bqkv_t[mi][:, mj : mj + 1])
            projS(wqkv[mi], HD,
                  lambda kt, p: src4[:, kt, 2 * p : 2 * p + 2, :], ev)
            return qe4

        for c in range(NP):
            bs = slice(c * N, (c + 1) * N)

            def load_x(xd):
                xb = T(xbp, [128, 8, N], BF16, "xb")
                for h in range(2):
                    xf = T(xinp, [128, 4, N], F32, "xin")
                    nc.sync.dma_start(out=xf[:], in_=xd[:, 4 * h : 4 * h + 4, bs])
                    nc.vector.tensor_copy(out=xb[:, 4 * h : 4 * h + 4, :],
                                          in_=xf[:])
                return xb

            xt_b = load_x(xt_d)
            xs_b = load_x(xs_d)

            def run_self(x_b, j, sum_dst, enh_dst):
                """expand -> self-MHA(j) -> +resid -> LN -> enh_dst [128,8,4,N];
                sum_dst [128,8,N] = sum_e enh."""
                xp = T(expp, [128, 8, 4, N], BF16, "exp")
                # expand: m-tile m = e*8+mj -> xp[:, mj, e, :]
                def ev_exp(mj0, gm, ps):
                    for mi in range(gm):
                        m = mj0 + mi
                        nc.scalar.activation(
                            xp[:, m % 8, m // 8, :], ps[mi][:, 0:256],
                            AF.Identity, bias=bexp_t[j][:, m : m + 1])
                projM(wexp[j], E * HD, HD, lambda kt: x_b[:, kt, :], ev_exp)

                k4, v4 = kv_proj(j, xp)
                qe4 = q_proj(j, xp)
                acc4 = T(acc4p, [128, 8, 4, N], BF16, "acc4")
                for e1 in range(4):
                    a_l = softmax_tiles(scores_all(qe4[:, :, e1, :], k4))
                    av_accum(a_l, v4, acc4[:, :, e1, :])
                # out proj with residual folded into PSUM; evict into enh_dst
                # (frees xp so the next phase's matmuls overlap the LN below)
                def ex_res(mj, p, ps):
                    nc.tensor.matmul(ps[:], i128[:],
                                     xp[:, mj, 2 * p : 2 * p + 2, :],
                                     start=False, stop=True)
                def ev_out(mj, ps):
                    for p in range(2):
                        nc.scalar.activation(
                            enh_dst[:, mj, 2 * p : 2 * p + 2, :], ps[p][:],
                            AF.Identity, bias=bout_t[j][:, mj : mj + 1])
                projS(wout[j], HD,
                      lambda kt, p: acc4[:, kt, 2 * p : 2 * p + 2, :],
                      ev_out, extra_mm=ex_res)
                for e1 in range(4):
                    ln_norm(enh_dst[:, :, e1, :], lng_t[j], lnb_t[j],
                            lambda mt, e1=e1: enh_dst[:, mt, e1, :])
                t2 = T(qkp, [128, 8, N], BF16, "qk")
                nc.vector.tensor_add(out=sum_dst[:], in0=enh_dst[:, :, 0, :],
                                     in1=enh_dst[:, :, 1, :])
                nc.vector.tensor_add(out=t2[:], in0=enh_dst[:, :, 2, :],
                                     in1=enh_dst[:, :, 3, :])
                nc.vector.tensor_add(out=sum_dst[:], in0=sum_dst[:], in1=t2[:])

            t_enh = T(enhp, [128, 8, 4, N], BF16, "enh")
            s_enh = T(enhp, [128, 8, 4, N], BF16, "enh")
            sum_t = T(actp, [128, 8, N], BF16, "act")
            sum_s = T(actp, [128, 8, N], BF16, "act")
            run_self(xt_b, 0, sum_t, t_enh)
            run_self(xs_b, 1, sum_s, s_enh)

            joint = T(actp, [128, 8, N], BF16, "act")
            projM(wjoint, HD, 2 * HD,
                  lambda kt: sum_t[:, kt, :] if kt < 8 else sum_s[:, kt - 8, :],
                  ev_split(lambda mj: joint[:, mj, :], bjoint_t, lambda mj: mj))

            def run_cross(mi, qsrc4, kvsrc4, dst):
                """cross-attn, output mean over query positions -> dst [128,8,N].
                wout[mi] pre-scaled by 0.25 on host."""
                k4, v4 = kv_proj(mi, kvsrc4)
                qe4 = q_proj(mi, qsrc4)
                abar = [None] * 4
                for e1 in range(4):
                    a_l = softmax_tiles(scores_all(qe4[:, :, e1, :], k4))
                    for e2 in range(4):
                        if e1 == 0:
                            ab = T(smabp, [8, N], F32, "smab")
                            nc.vector.tensor_copy(out=ab[:], in_=a_l[e2][:])
                            abar[e2] = ab
                        else:
                            nc.vector.tensor_add(out=abar[e2][:], in0=abar[e2][:],
                                                 in1=a_l[e2][:])
                abar_b = []
                for e2 in range(4):
                    ab = T(smbp, [8, N], BF16, "smb")
                    nc.vector.tensor_copy(out=ab[:], in_=abar[e2][:])
                    abar_b.append(ab)
                cacc = T(accp, [128, 8, N], BF16, "accx")
                av_accum(abar_b, v4, cacc[:])
                projM(wout[mi], HD, HD, lambda kt: cacc[:, kt, :],
                      ev_split(lambda mj: dst[:, mj, :], bout_t[mi],
                               lambda mj: mj))

            mts = T(meanp, [128, 8, N], BF16, "mean")
            mst = T(meanp, [128, 8, N], BF16, "mean")
            run_cross(2, t_enh, s_enh, mts)
            run_cross(3, s_enh, t_enh, mst)

            # mha4 group: mtj == msj == Wout4 @ (Wv4 @ joint + bv4) + bout4
            vj = T(actp, [128, 8, N], BF16, "act")
            projM(wqkv[4], HD, HD, lambda kt: joint[:, kt, :],
                  ev_split(lambda mj: vj[:, mj, :], bqkv_t[4],
                           lambda mj: 16 + mj), wcol0=2 * HD)
            mtj = T(meanp, [128, 8, N], BF16, "mean")
            projM(wout[4], HD, HD, lambda kt: vj[:, kt, :],
                  ev_split(lambda mj: mtj[:, mj, :], bout_t[4], lambda mj: mj))
            qj = T(actp, [128, 8, N], BF16, "act")
            projM(wqkv[4], HD, HD, lambda kt: joint[:, kt, :],
                  ev_split(lambda mj: qj[:, mj, :], bqkv_t[4], lambda mj: mj))

            # jx: single-query cross-attn (q = joint row), kv from t_enh/s_enh
            jacc = T(acc4p, [128, 8, 4, N], BF16, "acc4")
            for jj, enh in enumerate((t_enh, s_enh)):
                k4, v4 = kv_proj(4, enh)
                a_l = softmax_tiles(scores_all(qj[:], k4))
                av_accum(a_l, v4, jacc[:, :, jj, :])
            mjt = T(meanp, [128, 8, N], BF16, "mean")
            mjs = T(meanp, [128, 8, N], BF16, "mean")
            def ev_jx(mj, ps):
                for jj, dst in enumerate((mjt, mjs)):
                    nc.scalar.activation(
                        dst[:, mj, :], ps[0][:, jj * 256 : (jj + 1) * 256],
                        AF.Identity, bias=bout_t[4][:, mj : mj + 1])
            projS(wout[4], HD, lambda kt, p: jacc[:, kt, 0:2, :], ev_jx,
                  npair=1)

            # gates (sigmoid) + fused mults, interleaved to bound liveness
            def run_gate(g, in_a, in_b):
                gt = T(gatep, [128, 8, N], BF16, "gate")
                projM(wgate[g], HD, 2 * HD,
                      lambda kt: in_a[:, kt, :] if kt < 8 else in_b[:, kt - 8, :],
                      ev_split(lambda mj: gt[:, mj, :], bgate_t[g],
                               lambda mj: mj, func=AF.Sigmoid))
                return gt

            f2 = T(accp, [128, 8, N], BF16, "accx")
            gate_t = run_gate(0, mts, mtj)
            nc.vector.tensor_tensor(out=f2[:], in0=gate_t[:], in1=mtj[:],
                                    op=ALU.mult)
            nc.vector.tensor_tensor(out=mts[:], in0=gate_t[:], in1=mts[:],
                                    op=ALU.mult)
            gate_s = run_gate(1, mst, mtj)
            nc.vector.tensor_tensor(out=mst[:], in0=gate_s[:], in1=mst[:],
                                    op=ALU.mult)
            nc.vector.tensor_tensor(out=mtj[:], in0=gate_s[:], in1=mtj[:],
                                    op=ALU.mult)
            gate_j = run_gate(2, mjt, mjs)
            nc.vector.tensor_tensor(out=mjt[:], in0=gate_j[:], in1=mjt[:],
                                    op=ALU.mult)
            nc.vector.tensor_tensor(out=mjs[:], in0=gate_j[:], in1=mjs[:],
                                    op=ALU.mult)
            fs = [mts, mst, f2, mtj, mjt, mjs]

            h1 = T(expp, [128, 8, 4, N], BF16, "exp")
            def ev_h1(mj0, gm, ps):
                for mi in range(gm):
                    m = mj0 + mi
                    nc.scalar.activation(
                        h1[:, m % 8, m // 8, :], ps[mi][:, 0:256],
                        AF.Relu, bias=bo1_t[:, m : m + 1])
            projM(wo1, 2 * HD, 6 * HD, lambda kt: fs[kt // 8][:, kt % 8, :],
                  ev_h1)
            h2 = T(accp, [128, 8, N], BF16, "accx")
            projM(wo2, HD, 2 * HD, lambda kt: h1[:, kt % 8, kt // 8, :],
                  ev_split(lambda mj: h2[:, mj, :], bo2_t, lambda mj: mj))

            # final LN (g,b pre-scaled by res_w) + (1-res_w)/2*(xt+xs)
            yt = T(qkp, [128, 8, N], BF16, "qk")
            ln_norm(h2[:], lng_t[2], lnb_t[2], lambda mt: yt[:, mt, :])
            cres = (1.0 - res_w) * 0.5
            orig = T(qkp, [128, 8, N], BF16, "qk")
            nc.vector.tensor_add(out=orig[:], in0=xt_b[:], in1=xs_b[:])
            nc.vector.tensor_scalar_mul(orig[:], orig[:], cres)
            nc.vector.tensor_add(out=yt[:], in0=yt[:], in1=orig[:])
            nc.sync.dma_start(out=y_d[:, :, bs], in_=yt[:])

    nc.compile()
    return nc


def _sel_const():
    s = np.zeros((8, 8 * 128), np.float32)
    for mt in range(8):
        s[mt, mt * 128 : (mt + 1) * 128] = 1.0
    return s.astype(BF)


def _o32_const():
    o = np.zeros((128, 64), np.float32)
    o[:, 32] = 1.0
    return o.astype(BF)


def _wl(w):
    """torch-style [M_out, K_in] -> [128, K/128, M] bf16 (p = row in k-tile)."""
    a = np.asarray(w, np.float32).T          # [K, M]
    K, M = a.shape
    a = a.reshape(K // 128, 128, M).transpose(1, 0, 2)
    return np.ascontiguousarray(a).astype(BF)


def _prep_inputs(i):
    res_w = float(np.asarray(i["res_w"]).reshape(-1)[0])
    sc = 1.0 / math.sqrt(128.0)

    shared = {
        "wexp0": _wl(i["exp_t_w"]), "wexp1": _wl(i["exp_s_w"]),
        "bexp0": _bias_cols(np.asarray(i["exp_t_b"]) + np.asarray(i["pos_enc"]).reshape(-1)),
        "bexp1": _bias_cols(np.asarray(i["exp_s_b"]) + np.asarray(i["pos_enc"]).reshape(-1)),
        "wjoint": _wl(np.asarray(i["joint_w"], np.float32) * 0.25),
        "bjoint": _bias_cols(i["joint_b"]),
        "wo1": _wl(i["out1_w"]), "bo1": _bias_cols(i["out1_b"]),
        "wo2": _wl(i["out2_w"]), "bo2": _bias_cols(i["out2_b"]),
        "sel_c": _sel_const(), "o32_c": _o32_const(),
        "i128_c": np.eye(128, dtype=np.float32).astype(BF),
    }
    for g in range(3):
        shared[f"wgate{g}"] = _wl(i["gate_w"][g])
        shared[f"bgate{g}"] = _bias_cols(i["gate_b"][g])
    for m in range(5):
        w = np.asarray(i["mha_in_w"][m], np.float32).copy()
        b = np.asarray(i["mha_in_b"][m], np.float32).copy()
        w[:HD] *= sc
        b[:HD] *= sc
        shared[f"wqkv{m}"] = _wl(w)
        shared[f"bqkv{m}"] = _bias_cols(b)
        wo = np.asarray(i["mha_out_w"][m], np.float32)
        if m in (2, 3):
            wo = wo * 0.25      # fold mean over the 4 query positions
        shared[f"wout{m}"] = _wl(wo)
        shared[f"bout{m}"] = _bias_cols(i["mha_out_b"][m])
    for ln in range(3):
        g = np.asarray(i["ln_g"][ln], np.float32)
        b = np.asarray(i["ln_b"][ln], np.float32)
        if ln == 2:
            g = g * res_w
            b = b * res_w
        shared[f"lng{ln}"] = _bias_cols(g)
        shared[f"lnb{ln}"] = _bias_cols(b)

    def shard_x(x, c):
        xc = np.asarray(x, np.float32)[c * BC : (c + 1) * BC, 0, :]  # [512,1024]
        return np.ascontiguousarray(xc.T.reshape(8, 128, BC).transpose(1, 0, 2))

    in_maps = []
    for c in range(NCORES):
        m = dict(shared)
        m["xt"] = shard_x(i["temporal_features"], c)
        m["xs"] = shard_x(i["spatial_features"], c)
        in_maps.append(m)
    return res_w, in_maps


def kernel(**inputs):
    res_w, in_maps = _prep_inputs(inputs)
    nc = build(res_w)
    res = bass_utils.run_bass_kernel_spmd(nc, in_maps, core_ids=list(range(NCORES)))
    outs = []
    for c in range(NCORES):
        y = res.results[c]["y"]                                   # [128,8,512]
        outs.append(np.asarray(y).astype(np.float32)
                    .transpose(1, 0, 2).reshape(HD, BC).T)
    return np.concatenate(outs, 0)[:, None, :].astype(np.float32)
